# revision 1
# baseline (speedup 1.0000x reference)
"""Multi-head attention with QK-LayerNorm on 8 Trainium2 NeuronCores.

Problem: B=2, S=F=2048, D=1024, H=16, HD=64 (fp32).
    q = LN_head(x_q @ Wq) * HD^-0.5 ; k = LN_head(x_k @ Wk) ; v = x_v @ Wv
    ctx = softmax(q k^T) v ; out = LN(ctx) @ Wproj

Sharding (8 cores, 2 groups of 4 by batch):
    core c: batch b = c//4, row-slice r = c%4 (512 query rows, 512 kv rows).
    Each core computes q/k/v for its own 512 rows (all heads), all-gathers
    kT and v inside its 4-core group, then runs full attention + output
    projection for its 512 query rows.  No collective needed on the output.

Projections and the output matmul run in float32r (~1.5e-4 rel err, full
PE rate at N>=512).  The gathered k^T/v, the local q^T and the softmax
probabilities are bf16: this halves both all-gathers (<1 MB per rank ->
fast mesh collective regime) at ~2e-3 extra scale-relative error; all
matmul accumulation stays fp32 in PSUM.  Measured: ~460-550 us per
invocation (interleaved NEFF-unroll differencing; measurement noise
~+/-50 us), scale-relative max error ~3.7e-3.
"""

import numpy as np

import concourse.bass as bass
import concourse.mybir as mybir
import concourse.tile as tile
from concourse import bacc, bass_utils
from concourse.masks import make_identity

F32 = mybir.dt.float32
F32R = mybir.dt.float32r
BF16 = mybir.dt.bfloat16
AF = mybir.ActivationFunctionType

B, S, F, D, H, HD = 2, 2048, 2048, 1024, 16, 64
EPS = 1e-5
NCORES = 8
GP = 4                      # group size (cores per batch)
SL = S // GP                # 512 local query rows
FL = F // GP                # 512 local kv rows
KC = D // 128               # 8 D-chunks
MC = SL // 128              # 4 local row chunks
GROUPS = [[0, 1, 2, 3], [4, 5, 6, 7]]


def _dma_big(nc, out, in_):
    return nc.sync.dma_start(out=out, in_=in_)


def _dma_small(nc, out, in_):
    return nc.gpsimd.dma_start(out=out, in_=in_)


def _load_w(nc, pool, wparam):
    """Load a (D, D) weight into one SBUF tile (128, KC, D), fp32r."""
    w = pool.tile([128, KC, D], F32R, tag="w")
    _dma_big(nc, w, wparam.rearrange("(kc p) n -> p kc n", p=128).bitcast(F32R))
    return w


def _transpose_in(nc, tc, x_sb, xT, m, tp_psum, ident):
    """x_sb (128 rows=S-chunk m, D) fp32 -> xT[:, kc, 128m:128m+128] fp32r."""
    for half in range(2):
        ps = tp_psum.tile([128, 512], F32, tag="tp")
        for q in range(4):
            kc = half * 4 + q
            nc.tensor.transpose(
                ps[:, q * 128:(q + 1) * 128], x_sb[:, kc * 128:(kc + 1) * 128],
                ident[:, :],
            )
        out_view = xT[:, half * 4:(half + 1) * 4, m * 128:(m + 1) * 128]
        in_view = ps[:, :].rearrange("p (q j) -> p q j", j=128)
        nc.vector.tensor_copy(out_view, in_view)


def build(n_repeat=1):
    nc = bacc.Bacc(None, target_bir_lowering=False)

    xq = nc.declare_dram_parameter("xq", [SL, D], F32, isOutput=False)
    xk = nc.declare_dram_parameter("xk", [FL, D], F32, isOutput=False)
    xv = nc.declare_dram_parameter("xv", [FL, D], F32, isOutput=False)
    wq = nc.declare_dram_parameter("wq", [D, D], F32, isOutput=False)
    wk = nc.declare_dram_parameter("wk", [D, D], F32, isOutput=False)
    wv = nc.declare_dram_parameter("wv", [D, D], F32, isOutput=False)
    wp = nc.declare_dram_parameter("wp", [D, D], F32, isOutput=False)
    qg = nc.declare_dram_parameter("qg", [HD], F32, isOutput=False)  # *scale
    qb = nc.declare_dram_parameter("qb", [HD], F32, isOutput=False)  # *scale
    kg = nc.declare_dram_parameter("kg", [HD], F32, isOutput=False)
    kb = nc.declare_dram_parameter("kb", [HD], F32, isOutput=False)
    og = nc.declare_dram_parameter("og", [D], F32, isOutput=False)
    ob = nc.declare_dram_parameter("ob", [D], F32, isOutput=False)
    out = nc.declare_dram_parameter("out", [SL, D], F32, isOutput=True)

    with tile.TileContext(nc) as tc:
        with (
            tc.tile_pool(name="const", bufs=1) as const,
            tc.tile_pool(name="persist", bufs=1) as persist,
            tc.tile_pool(name="dram", bufs=1, space="DRAM") as dram,
        ):
            # ---- constants ----
            ident = const.tile([128, 128], F32)
            make_identity(nc, ident[:, :])
            ones_f = const.tile([128, 1], F32)
            nc.vector.memset(ones_f, 1.0)
            ones_col = const.tile([128, 1], F32R)
            nc.vector.tensor_copy(ones_col, ones_f)
            eps_t = const.tile([128, 1], F32)
            nc.vector.memset(eps_t, EPS)
            ones_row_f = const.tile([1, 128], F32)
            nc.vector.memset(ones_row_f, 1.0)
            ones_row = const.tile([1, 128], F32R)
            nc.vector.tensor_copy(ones_row, ones_row_f)

            def bcast_gb(param):
                t = const.tile([128, HD], F32, tag=f"gb_{param.name}")
                _dma_small(nc, t, param[None, :].to_broadcast([128, HD]))
                return t

            qg_t = bcast_gb(qg)
            qb_t = bcast_gb(qb)
            kg_t = bcast_gb(kg)
            kb_t = bcast_gb(kb)

            og_pp = const.tile([128, KC], F32)
            _dma_small(nc, og_pp, og.rearrange("(kc p) -> p kc", p=128))
            ob_pp = const.tile([128, KC], F32)
            _dma_small(nc, ob_pp, ob.rearrange("(kc p) -> p kc", p=128))

            # (loop over n_repeat for benchmarking; n_repeat=1 in production)
            for _rep in range(n_repeat):
                # ---- persistent SBUF arrays ----
                qT = persist.tile([128, KC, SL], BF16, tag="qT", name="qT")
                ctxT = persist.tile([128, KC, SL], F32R, tag="ctxT", name="ctxT")

                # ---- DRAM bounce / gathered tensors ----
                kT_bounce = dram.tile([D, FL], BF16, tag="kTb", name="kTb")
                v_bounce = dram.tile([FL, D], BF16, tag="vb", name="vb")
                kT_all = dram.tile([GP, D, FL], BF16, tag="kTa", name="kTa")
                v_all = dram.tile([GP, FL, D], BF16, tag="va", name="va")

                # ================= phase 1: projections =================
                _wpool_cm = tc.tile_pool(name="wpool", bufs=2)
                wpool = _wpool_cm.__enter__()
                _xpool_cm = tc.tile_pool(name="xpool", bufs=6)
                xpool = _xpool_cm.__enter__()

                def proj_chain(xparam, w, which):
                    """Projection for one of q/k/v from a preloaded weight tile.

                    'q'/'k': LN + transpose into qT / kT_bounce; 'v': natural.
                    LN+transpose of m-chunk m-1 is emitted after m's matmuls so
                    the PE never stalls waiting for the DVE layernorm.
                    """
                    with (
                        tc.tile_pool(name=f"p1_{which}", bufs=2) as p1,
                        tc.tile_pool(name=f"p1x_{which}", bufs=1) as p1x,
                        tc.tile_pool(name=f"p1ps_{which}", bufs=2, space="PSUM") as p1ps,
                        tc.tile_pool(name=f"p1tp_{which}", bufs=2, space="PSUM") as p1tp,
                    ):
                        xT = p1x.tile([128, KC, SL], F32R, tag="xT", name="xT")
                        for m in range(MC):
                            x_sb = xpool.tile([128, D], F32, tag="x", name="x_sb")
                            _dma_big(nc, x_sb, xparam[m * 128:(m + 1) * 128, :])
                            _transpose_in(nc, tc, x_sb, xT, m, p1tp, ident)

                        def mm_stage(m):
                            ps = p1ps.tile([128, D], F32, tag="nat", name="nat")
                            for n in range(2):
                                for kc in range(KC):
                                    nc.tensor.matmul(
                                        ps[:, n * 512:(n + 1) * 512],
                                        xT[:, kc, m * 128:(m + 1) * 128],
                                        w[:, kc, n * 512:(n + 1) * 512],
                                        start=(kc == 0), stop=(kc == KC - 1),
                                    )
                            return ps

                        def post_stage(m, ps):
                            if which == "v":
                                v_sb = p1.tile([128, D], BF16, tag="vout",
                                               name="v_sb")
                                nc.vector.tensor_copy(v_sb, ps[:, :])
                                _dma_big(nc, v_bounce[m * 128:(m + 1) * 128, :],
                                         v_sb)
                                return
                            g_t, b_t = (qg_t, qb_t) if which == "q" else (kg_t, kb_t)
                            ln = p1.tile([128, H, HD], F32, tag="ln", name="ln")
                            stats = p1.tile([128, H, 6], F32, tag="st", name="st")
                            mv = p1.tile([128, H, 2], F32, tag="mv", name="mv")
                            psv = ps[:, :].rearrange("p (h d) -> p h d", d=HD)
                            for h in range(H):
                                nc.vector.bn_stats(stats[:, h, :], psv[:, h, :])
                                nc.vector.bn_aggr(mv[:, h, :], stats[:, h, :])
                            # rstd = 1/sqrt(var+eps)
                            nc.scalar.activation(
                                mv[:, :, 1], mv[:, :, 1], AF.Sqrt,
                                bias=eps_t[:, :])
                            nc.vector.reciprocal(mv[:, :, 1], mv[:, :, 1])
                            for h in range(H):
                                nc.vector.tensor_scalar(
                                    out=ln[:, h, :], in0=psv[:, h, :],
                                    scalar1=mv[:, h, 0:1], scalar2=mv[:, h, 1:2],
                                    op0=mybir.AluOpType.subtract,
                                    op1=mybir.AluOpType.mult,
                                )
                            nc.vector.tensor_mul(
                                ln, ln, g_t[:, None, :].broadcast_to([128, H, HD]))
                            nc.vector.tensor_add(
                                ln, ln, b_t[:, None, :].broadcast_to([128, H, HD]))
                            lnf = ln.rearrange("p h d -> p (h d)")
                            if which == "q":
                                _transpose_in(nc, tc, lnf, qT, m, p1tp, ident)
                            else:
                                for half in range(2):
                                    psT = p1tp.tile([128, 512], F32, tag="tp",
                                                    name="psT")
                                    for qq in range(4):
                                        kc = half * 4 + qq
                                        nc.tensor.transpose(
                                            psT[:, qq * 128:(qq + 1) * 128],
                                            lnf[:, kc * 128:(kc + 1) * 128],
                                            ident[:, :])
                                    kTl = p1.tile([128, 4, 128], BF16, tag="kTl",
                                                  name="kTl")
                                    nc.vector.tensor_copy(
                                        kTl,
                                        psT[:, :].rearrange("p (q j) -> p q j", j=128))
                                    _dma_big(
                                        nc,
                                        kT_bounce.rearrange(
                                            "(kc p) s -> p kc s", p=128)
                                        [:, half * 4:(half + 1) * 4,
                                         m * 128:(m + 1) * 128],
                                        kTl)

                        prev = None
                        for m in range(MC):
                            ps = mm_stage(m)
                            if prev is not None:
                                post_stage(m - 1, prev)
                            prev = ps
                        post_stage(MC - 1, prev)

                w_k = _load_w(nc, wpool, wk)
                w_v = _load_w(nc, wpool, wv)
                proj_chain(xk, w_k, "k")
                nc.gpsimd.collective_compute(
                    "AllGather", mybir.AluOpType.bypass, replica_groups=GROUPS,
                    ins=[kT_bounce.opt()], outs=[kT_all.opt()],
                )
                w_q = _load_w(nc, wpool, wq)
                proj_chain(xv, w_v, "v")
                nc.gpsimd.collective_compute(
                    "AllGather", mybir.AluOpType.bypass, replica_groups=GROUPS,
                    ins=[v_bounce.opt()], outs=[v_all.opt()],
                )
                w_p = _load_w(nc, wpool, wp)
                proj_chain(xq, w_q, "q")
                _xpool_cm.__exit__(None, None, None)

                # ================= phase 2: attention =================
                with (
                    tc.tile_pool(name="vext", bufs=1) as vextp,
                    tc.tile_pool(name="att", bufs=3) as att,
                    tc.tile_pool(name="kpair", bufs=2) as kpairp,
                    tc.tile_pool(name="att_ps", bufs=3, space="PSUM") as att_ps,
                    tc.tile_pool(name="ctx_ps", bufs=1, space="PSUM") as ctx_psp,
                ):
                    # v_ext: 16 tiles (128, H, HD+1), col HD is ones
                    v_ext = []
                    for j in range(F // 128):
                        g, lj = j // 4, j % 4
                        vt = vextp.tile([128, H, HD + 1], BF16, tag=f"vext{j}")
                        _dma_big(
                            nc, vt[:, :, 1:HD + 1],
                            v_all[g, lj * 128:(lj + 1) * 128, :]
                            .rearrange("p (h d) -> p h d", d=HD))
                        nc.vector.tensor_copy(
                            vt[:, :, 0:1],
                            ones_f[:, None, :].broadcast_to([128, H, 1]))
                        v_ext.append(vt)

                    for j in range(H // 2):      # head pairs
                        kT_pair = kpairp.tile([128, F], BF16, tag="kp")
                        for g in range(GP):
                            _dma_big(
                                nc, kT_pair[:, g * FL:(g + 1) * FL],
                                kT_all[g, j * 128:(j + 1) * 128, :])
                        ctx_ps = [
                            ctx_psp.tile([HD + 1, SL], F32, tag="ctxA", name="ctxA"),
                            ctx_psp.tile([HD + 1, SL], F32, tag="ctxB", name="ctxB"),
                        ]
                        for sc in range(8):      # supers of 2 F-chunks
                            sp = [None, None]
                            pt = [None, None]
                            for hh in range(2):
                                sp[hh] = att_ps.tile([128, 1024], F32, tag="sp", name="sp")
                            for cc in range(2):
                                fc = sc * 2 + cc
                                for hh in range(2):
                                    nc.tensor.matmul(
                                        sp[hh][:, cc * 512:(cc + 1) * 512],
                                        kT_pair[hh * 64:(hh + 1) * 64,
                                                fc * 128:(fc + 1) * 128],
                                        qT[hh * 64:(hh + 1) * 64, j, :],
                                        start=True, stop=True,
                                        tile_position=(hh * 64, 0),
                                    )
                            for hh in range(2):
                                pt[hh] = att.tile([128, 1024], BF16, tag="pt", name="pt")
                                nc.scalar.activation(pt[hh], sp[hh][:, :], AF.Exp)
                            for cc in range(2):
                                fc = sc * 2 + cc
                                for hh in range(2):
                                    nc.tensor.matmul(
                                        ctx_ps[hh][:, :],
                                        v_ext[fc][:, 2 * j + hh, :],
                                        pt[hh][:, cc * 512:(cc + 1) * 512],
                                        start=(sc == 0 and cc == 0),
                                        stop=(sc == 7 and cc == 1),
                                    )
                        # normalize by softmax denominator (row 0 of ctx_ps):
                        # recip (DVE) -> rank-1 PE broadcast to 65 partitions ->
                        # copy psum->sbuf (DVE) -> multiply (DVE) -> SBUF DMA
                        # moves rows into ctxT partitions.
                        for hh in range(2):
                            den_r = att.tile([1, SL], F32R, tag="den_r")
                            with nc.allow_low_precision(
                                    reason="fp32r is 32-bit storage"):
                                nc.vector.reciprocal(den_r, ctx_ps[hh][0:1, :])
                            bc_ps = att_ps.tile(
                                [HD + 1, SL], F32, tag="sp", name="bc_ps")
                            nc.tensor.matmul(
                                bc_ps[:, :], ones_row[:, 0:HD + 1], den_r,
                                start=True, stop=True)
                            rbc = att.tile([HD + 1, SL], F32, tag="rbc")
                            nc.vector.tensor_copy(rbc, bc_ps[:, :])
                            tmp = att.tile([HD + 1, SL], F32, tag="ctmp")
                            nc.vector.tensor_mul(tmp, ctx_ps[hh][:, :], rbc)
                            _dma_big(
                                nc, ctxT[hh * 64:(hh + 1) * 64, j, :],
                                tmp[1:HD + 1, :].bitcast(F32R))

                # ================= phase 3: out-LN + projection =================
                with (
                    tc.tile_pool(name="p3", bufs=2) as p3,
                    tc.tile_pool(name="p3w", bufs=1) as p3w,
                    tc.tile_pool(name="p3s", bufs=1) as p3s,
                    tc.tile_pool(name="st_ps", bufs=1, space="PSUM") as st_ps,
                    tc.tile_pool(name="o_ps", bufs=2, space="PSUM") as o_ps,
                ):
                    wproj = w_p
                    sum_ps = st_ps.tile([1, SL], F32, tag="sum")
                    for kc in range(KC):
                        nc.tensor.matmul(sum_ps[:, :], ones_col, ctxT[:, kc, :],
                                         start=(kc == 0), stop=(kc == KC - 1))
                    sq_ps = st_ps.tile([1, SL], F32, tag="sq")
                    for kc in range(KC):
                        sq = p3.tile([128, SL], F32R, tag="sq")
                        nc.vector.tensor_mul(
                            sq, ctxT[:, kc, :].bitcast(F32), ctxT[:, kc, :].bitcast(F32))
                        nc.tensor.matmul(sq_ps[:, :], ones_col, sq,
                                         start=(kc == 0), stop=(kc == KC - 1))
                    mean = p3s.tile([1, SL], F32, tag="mean")
                    nc.vector.tensor_scalar_mul(mean, sum_ps[:, :], 1.0 / D)
                    ex2 = p3s.tile([1, SL], F32, tag="ex2")
                    nc.vector.tensor_scalar_mul(ex2, sq_ps[:, :], 1.0 / D)
                    m2 = p3s.tile([1, SL], F32, tag="m2")
                    nc.vector.tensor_mul(m2, mean, mean)
                    var = p3s.tile([1, SL], F32, tag="var")
                    nc.vector.tensor_sub(var, ex2, m2)
                    nc.scalar.activation(var, var, AF.Sqrt, bias=eps_t[0:1, :])
                    rstd = p3s.tile([1, SL], F32R, tag="rstd")
                    negm = p3s.tile([1, SL], F32R, tag="negm")
                    with nc.allow_low_precision(
                            reason="fp32r is 32-bit storage"):
                        nc.vector.reciprocal(rstd, var)
                        nc.vector.tensor_mul(negm, mean, rstd.bitcast(F32))
                        nc.vector.tensor_scalar_mul(negm, negm, -1.0)
                    rstd_ps = st_ps.tile([128, SL], F32, tag="rstd_ps")
                    nc.tensor.matmul(rstd_ps[:, :], ones_row, rstd,
                                     start=True, stop=True)
                    negm_ps = st_ps.tile([128, SL], F32, tag="negm_ps")
                    nc.tensor.matmul(negm_ps[:, :], ones_row, negm,
                                     start=True, stop=True)

                    ctxn = p3w.tile([128, KC, SL], F32R, tag="ctxn")
                    for kc in range(KC):
                        t = p3.tile([128, SL], F32, tag="lnt")
                        nc.vector.tensor_mul(
                            t, ctxT[:, kc, :].bitcast(F32), rstd_ps[:, :])
                        nc.vector.tensor_add(t, t, negm_ps[:, :])
                        nc.vector.tensor_scalar(
                            out=ctxn[:, kc, :], in0=t,
                            scalar1=og_pp[:, kc:kc + 1], scalar2=ob_pp[:, kc:kc + 1],
                            op0=mybir.AluOpType.mult, op1=mybir.AluOpType.add)

                    for m in range(MC):
                        ps = o_ps.tile([128, D], F32, tag="o")
                        for n in range(2):
                            for kc in range(KC):
                                nc.tensor.matmul(
                                    ps[:, n * 512:(n + 1) * 512],
                                    ctxn[:, kc, m * 128:(m + 1) * 128],
                                    wproj[:, kc, n * 512:(n + 1) * 512],
                                    start=(kc == 0), stop=(kc == KC - 1),
                                )
                        o_sb = p3.tile([128, D], F32, tag="osb")
                        nc.vector.tensor_copy(o_sb, ps[:, :])
                        _dma_big(nc, out[m * 128:(m + 1) * 128, :], o_sb)

                _wpool_cm.__exit__(None, None, None)
    nc.finalize()
    return nc


_NC_CACHE = None


def _get_nc(n_repeat=1):
    global _NC_CACHE
    if _NC_CACHE is None:
        _NC_CACHE = build(n_repeat)
    return _NC_CACHE


def kernel(x_q, x_k, x_v, Wq, Wk, Wv, Wproj,
           q_gamma, q_beta, k_gamma, k_beta, out_gamma, out_beta,
           _trace=False):
    scale = np.float32(HD ** -0.5)
    x_q = np.ascontiguousarray(np.asarray(x_q, dtype=np.float32))
    x_k = np.ascontiguousarray(np.asarray(x_k, dtype=np.float32))
    x_v = np.ascontiguousarray(np.asarray(x_v, dtype=np.float32))
    common = {
        "wq": np.ascontiguousarray(np.asarray(Wq, np.float32)),
        "wk": np.ascontiguousarray(np.asarray(Wk, np.float32)),
        "wv": np.ascontiguousarray(np.asarray(Wv, np.float32)),
        "wp": np.ascontiguousarray(np.asarray(Wproj, np.float32)),
        "qg": np.asarray(q_gamma, np.float32) * scale,
        "qb": np.asarray(q_beta, np.float32) * scale,
        "kg": np.asarray(k_gamma, np.float32),
        "kb": np.asarray(k_beta, np.float32),
        "og": np.asarray(out_gamma, np.float32),
        "ob": np.asarray(out_beta, np.float32),
    }
    in_maps = []
    for c in range(NCORES):
        b, r = c // GP, c % GP
        in_maps.append({
            "xq": x_q[b, r * SL:(r + 1) * SL, :],
            "xk": x_k[b, r * FL:(r + 1) * FL, :],
            "xv": x_v[b, r * FL:(r + 1) * FL, :],
            **common,
        })
    nc = _get_nc()
    res = bass_utils.run_bass_kernel_spmd(
        nc, in_maps, list(range(NCORES)), trace=_trace)
    full = np.empty((B, S, D), dtype=np.float32)
    for c in range(NCORES):
        b, r = c // GP, c % GP
        full[b, r * SL:(r + 1) * SL, :] = res.results[c]["out"]
    if _trace:
        return full, res
    return full

